# revision 2
# baseline (speedup 1.0000x reference)
import sys

sys.path.insert(0, "/opt/trn_rl_repo")
import numpy as np
import ml_dtypes
import concourse.mybir as mybir
from concourse import bacc
from concourse.tile import TileContext
from concourse.bass_utils import run_bass_kernel_spmd

F32 = mybir.dt.float32
F32R = mybir.dt.float32r
BF16 = mybir.dt.bfloat16
EXP = mybir.ActivationFunctionType.Exp

B, S, D = 4, 2048, 1024
NH, HD = 16, 64
USE_F32R = True


def build(use_f32r=USE_F32R):
    DT = F32R if use_f32r else BF16
    nc = bacc.Bacc()
    qx = nc.declare_dram_parameter("qx", [128, 8, 2048], BF16, isOutput=False)
    kx = nc.declare_dram_parameter("kx", [128, 8, 2048], BF16, isOutput=False)
    vx = nc.declare_dram_parameter("vx", [128, 8, 2048], BF16, isOutput=False)
    wq = nc.declare_dram_parameter("wq", [128, 8, 512], BF16, isOutput=False)
    wk = nc.declare_dram_parameter("wk", [128, 8, 512], BF16, isOutput=False)
    wv = nc.declare_dram_parameter("wv", [128, 8, 512], BF16, isOutput=False)
    wo = nc.declare_dram_parameter("wo", [128, 8, 512], BF16, isOutput=False)
    yT = nc.declare_dram_parameter("yT", [128, 8, 2048], BF16, isOutput=True)

    with TileContext(nc) as tc:
        with tc.sbuf_pool(name="sb", bufs=1) as pool, tc.psum_pool(
            name="ps", bufs=1
        ) as pp:
            # ---- persistent SBUF tiles ----
            # weight tiles (held for the whole kernel)
            wq_sb = pool.tile([128, 8, 512], BF16, tag="wqs")
            wk_sb = pool.tile([128, 8, 512], BF16, tag="wks")
            wv_sb = pool.tile([128, 8, 512], BF16, tag="wvs")
            wo_sb = pool.tile([128, 8, 512], BF16, tag="wo")

            # v_sb[p, kti, h*65 : h*65+64] = V^T values; column h*65+64 stays 1.0
            # (ones column makes the PV matmul also accumulate softmax denoms)
            v_sb = pool.tile([128, 16, 520], BF16, tag="vsb")
            nc.vector.memset(v_sb[:], 1.0)

            qt = [
                pool.tile([128, 2048], DT, tag=f"qt{r}", name=f"qt{r}")
                for r in range(4)
            ]
            kt = [
                pool.tile([128, 2048], DT, tag=f"kt{r}", name=f"kt{r}")
                for r in range(4)
            ]

            # ---- projection helpers ----
            # input chunk: [128, 8, 512] = all 1024 in-dims x 512 seq positions
            def load_chunk(xin, sl):
                i_t = pool.tile([128, 8, 512], BF16, tag="inb", bufs=3)
                nc.sync.dma_start(
                    out=i_t[:], in_=xin[:, :, sl * 512 : (sl + 1) * 512]
                )
                return i_t

            def proj_qk_slice(i_t, w_sb, out_tiles, sl, r, on_act):
                # one r-block (128 out dims) for one seq slice (512 positions)
                big = pp.tile([128, 1024], F32, tag="big", bufs=2)
                half = sl % 2
                dst = big[:, half * 512 : (half + 1) * 512]
                for kc in range(8):
                    nc.tensor.matmul(
                        dst,
                        w_sb[:, kc, r * 128 : (r + 1) * 128],
                        i_t[:, kc, :],
                        start=(kc == 0),
                        stop=(kc == 7),
                    )
                out_ap = out_tiles[r][:, sl * 512 : (sl + 1) * 512]
                if on_act:
                    nc.scalar.copy(out=out_ap, in_=dst)
                else:
                    nc.vector.tensor_copy(out=out_ap, in_=dst)

            def proj_v_kti(i_t, kti):
                # V^T for one 128-wide k chunk: out partitions = kseq,
                # free = 8 heads x 64, interleaved into v_sb with stride 65
                big = pp.tile([128, 1024], F32, tag="big", bufs=2)
                half = kti % 2
                dst = big[:, half * 512 : (half + 1) * 512]
                ktl = kti % 4
                for kc in range(8):
                    nc.tensor.matmul(
                        dst,
                        i_t[:, kc, ktl * 128 : (ktl + 1) * 128],
                        wv_sb[:, kc, :],
                        start=(kc == 0),
                        stop=(kc == 7),
                    )
                # strided copy: 8 heads of 64, skipping the ones column
                nc.scalar.copy(
                    out=v_sb[:, kti, 0:520].rearrange("p (h c) -> p h c", c=65)[
                        :, :, 0:64
                    ],
                    in_=dst.rearrange("p (h c) -> p h c", c=64),
                )

            # ---- phase 1: K full, V full, Q slice 0 ----
            # DMA order: first K chunk + wk first so the first matmul can
            # start as early as possible; remaining weights follow.
            k_it0 = pool.tile([128, 8, 512], BF16, tag="inb", bufs=3)
            nc.sync.dma_start(out=k_it0[:, 0:4, :], in_=kx[:, 0:4, 0:512])
            nc.sync.dma_start(out=wk_sb[:, 0:4, :], in_=wk[:, 0:4, :])
            nc.sync.dma_start(out=k_it0[:, 4:8, :], in_=kx[:, 4:8, 0:512])
            nc.sync.dma_start(out=wk_sb[:, 4:8, :], in_=wk[:, 4:8, :])
            k_it1 = load_chunk(kx, 1)
            nc.sync.dma_start(out=wv_sb[:], in_=wv[:])
            nc.sync.dma_start(out=wq_sb[:], in_=wq[:])
            nc.sync.dma_start(out=wo_sb[:], in_=wo[:])
            # first slice: half-contraction groups so matmuls start early
            sl0_bigs = []
            for r in range(4):
                big = pp.tile([128, 1024], F32, tag="big", bufs=2)
                dst = big[:, 0:512] if r % 2 == 0 else big[:, 512:1024]
                for kc in range(4):
                    nc.tensor.matmul(
                        dst,
                        wk_sb[:, kc, r * 128 : (r + 1) * 128],
                        k_it0[:, kc, :],
                        start=(kc == 0),
                        stop=False,
                    )
                sl0_bigs.append((big, dst))
            for r in range(4):
                big, dst = sl0_bigs[r]
                for kc in range(4, 8):
                    nc.tensor.matmul(
                        dst,
                        wk_sb[:, kc, r * 128 : (r + 1) * 128],
                        k_it0[:, kc, :],
                        start=False,
                        stop=(kc == 7),
                    )
                nc.scalar.copy(out=kt[r][:, 0:512], in_=dst)
            for sl in range(1, 4):
                i_t = k_it1 if sl == 1 else load_chunk(kx, sl)
                for r in range(4):
                    proj_qk_slice(i_t, wk_sb, kt, sl, r, on_act=True)
            for sl in range(4):
                i_t = load_chunk(vx, sl)
                for k4 in range(4):
                    proj_v_kti(i_t, sl * 4 + k4)
            q_it = [load_chunk(qx, sl) for sl in range(1)]
            for r in range(4):
                proj_qk_slice(q_it[0], wq_sb, qt, 0, r, on_act=True)

            # ---- filler emitters (run between attention streams) ----
            fillers = []  # list of callables, each ~one PE group

            def mk_qproj_filler(sl, r):
                def f(i_t=None, sl=sl, r=r):
                    proj_qk_slice(qproj_its[sl], wq_sb, qt, sl, r, on_act=False)

                return f

            qproj_its = {}

            ot_store = {}  # (qb, r) -> ot tile

            def mk_outproj_filler(qb, dmc):
                def f(qb=qb, dmc=dmc):
                    big = pp.tile([128, 1024], F32, tag="big", bufs=2)
                    dst = big[:, (dmc % 2) * 512 : (dmc % 2) * 512 + 512]
                    for r in range(4):
                        nc.tensor.matmul(
                            dst,
                            wo_sb[
                                :,
                                2 * r + dmc // 4,
                                (dmc % 4) * 128 : (dmc % 4) * 128 + 128,
                            ],
                            ot_store[(qb, r)][:],
                            start=(r == 0),
                            stop=(r == 3),
                        )
                    yb = pool.tile([128, 512], BF16, tag="yb", bufs=3)
                    nc.vector.tensor_copy(out=yb[:], in_=dst)
                    nc.sync.dma_start(
                        out=yT[:, dmc, qb * 512 : (qb + 1) * 512], in_=yb[:]
                    )

                return f

            # ---- attention stream for one (qb, r): 2 heads ----
            def stream(qb, r, tail_fillers=()):
                acc = pp.tile([128, 1024], F32, tag="acc", bufs=2)
                pts = {}

                def do_exp(k, big_p):
                    pt = pool.tile([128, 1024], BF16, tag="pt", bufs=3)
                    nc.scalar.activation(
                        out=pt[:], in_=big_p[:], func=EXP, scale=0.125
                    )
                    pts[k] = pt

                def do_pv(k):
                    pt = pts.pop(k)
                    for h in range(2):
                        nc.tensor.matmul(
                            acc[0:65, h * 512 : (h + 1) * 512],
                            v_sb[:, k, (2 * r + h) * 65 : (2 * r + h) * 65 + 65],
                            pt[:, h * 512 : (h + 1) * 512],
                            start=(k == 0),
                            stop=(k == 15),
                        )

                # PV lags exp by 2 k-tiles so the PE never waits on the
                # freshest activation (+ its semaphore hop)
                n_fill = len(tail_fillers)
                mids = {}
                if n_fill == 2:
                    mids = {8: tail_fillers[0]}
                elif n_fill >= 3:
                    mids = {6: tail_fillers[0], 11: tail_fillers[1]}
                tail = tail_fillers[max(0, n_fill - 1) :] if n_fill else ()
                big_prev = None
                for kti in range(16):
                    if kti in mids:
                        mids[kti]()
                    big = pp.tile([128, 1024], F32, tag="big", bufs=2)
                    nc.tensor.matmul(
                        big[:, 0:512],
                        kt[r][0:64, kti * 128 : (kti + 1) * 128],
                        qt[r][0:64, qb * 512 : (qb + 1) * 512],
                        start=True,
                        stop=True,
                    )
                    nc.tensor.matmul(
                        big[:, 512:1024],
                        kt[r][64:128, kti * 128 : (kti + 1) * 128],
                        qt[r][64:128, qb * 512 : (qb + 1) * 512],
                        start=True,
                        stop=True,
                    )
                    if big_prev is not None:
                        do_exp(kti - 1, big_prev)
                    big_prev = big
                    if kti >= 2:
                        do_pv(kti - 2)
                do_exp(15, big_prev)
                # last filler PE group runs here while ACT finishes the
                # last two exps of this stream
                for f in tail:
                    f()
                do_pv(14)
                do_pv(15)

                # normalize: row 64 of acc holds softmax denominators
                rec = pool.tile([1, 1024], BF16, tag="rec", bufs=2)
                with nc.allow_low_precision(reason="softmax denom recip bf16"):
                    nc.vector.reciprocal(out=rec[:], in_=acc[64:65, :])
                bc = pool.tile([128, 1024], BF16, tag="bc", bufs=2)
                nc.gpsimd.partition_broadcast(bc[:, 0:512], rec[0:1, 0:512])
                nc.gpsimd.partition_broadcast(bc[:, 512:1024], rec[0:1, 512:1024])
                ot = pool.tile([128, 512], BF16, tag="ot", bufs=9)
                nc.vector.tensor_mul(
                    out=ot[0:64, :], in0=acc[0:64, 0:512], in1=bc[0:64, 0:512]
                )
                nc.vector.tensor_mul(
                    out=ot[64:128, :], in0=acc[0:64, 512:1024], in1=bc[64:128, 512:1024]
                )
                ot_store[(qb, r)] = ot

            # ---- filler schedule ----
            # qb0 slots: Q proj slices 1,2 (2 groups/slot)
            # qb1 slots: Q proj slice 3 + outproj qb0 (3 groups/slot)
            # qb2 slots: outproj qb1 (2 groups/slot)
            # qb3 slots: outproj qb2 (2 groups/slot); tail: outproj qb3
            for sl in (1, 2, 3):
                qproj_its[sl] = None  # loaded lazily below

            slot_fillers = {qb: [[] for _ in range(4)] for qb in range(4)}
            for i, (sl, r) in enumerate(
                [(sl, r) for sl in (1, 2) for r in range(4)]
            ):
                slot_fillers[0][i // 2].append(mk_qproj_filler(sl, r))
            for i, r in enumerate(range(4)):
                slot_fillers[1][i].append(mk_qproj_filler(3, r))
            for qb_src, qb_host in ((0, 1), (1, 2), (2, 3)):
                for dmc in range(8):
                    slot_fillers[qb_host][dmc // 2].append(
                        mk_outproj_filler(qb_src, dmc)
                    )
            # spread each slot's groups across the stream (k-tiles 6, 11, tail)

            # ---- main attention loop ----
            for qb in range(4):
                # issue DMA for Q slices needed by upcoming fillers
                if qb == 0:
                    qproj_its[1] = load_chunk(qx, 1)
                    qproj_its[2] = load_chunk(qx, 2)
                if qb == 1:
                    qproj_its[3] = load_chunk(qx, 3)
                for r in range(4):
                    stream(qb, r, slot_fillers[qb][r])
            for dmc in range(8):
                mk_outproj_filler(3, dmc)()
    return nc


def _pack_in(x):  # [2048, 1024] -> [128, 8, 2048]
    return np.ascontiguousarray(x.T.reshape(8, 128, 2048).transpose(1, 0, 2))


def _pack_w(wt, g):  # W.T [1024,1024] cols for group g -> [128, 8, 512]
    return np.ascontiguousarray(
        wt[:, 512 * g : 512 * (g + 1)].reshape(8, 128, 512).transpose(1, 0, 2)
    )


def _pack_wo(wot, g):  # Wo.T rows for group g -> [128, 8, 512] bf16
    a = wot[512 * g : 512 * (g + 1), :].reshape(4, 128, 1024).transpose(1, 0, 2)
    w8 = np.empty((128, 8, 512), np.float32)
    for r in range(4):
        for j in range(2):
            w8[:, 2 * r + j, :] = a[:, r, j * 512 : (j + 1) * 512]
    return w8.astype(ml_dtypes.bfloat16)


def _prepare(inputs):
    query = np.asarray(inputs["query"], np.float32)
    key = np.asarray(inputs["key"], np.float32)
    value = np.asarray(inputs["value"], np.float32)
    WqT = np.asarray(inputs["Wq"], np.float32).T
    WkT = np.asarray(inputs["Wk"], np.float32).T
    WvT = np.asarray(inputs["Wv"], np.float32).T
    WoT = np.asarray(inputs["Wo"], np.float32).T

    cast = lambda a: a.astype(ml_dtypes.bfloat16)
    in_maps = []
    for c in range(8):
        b, g = c // 2, c % 2
        in_maps.append(
            {
                "qx": cast(_pack_in(query[b])),
                "kx": cast(_pack_in(key[b])),
                "vx": cast(_pack_in(value[b])),
                "wq": cast(_pack_w(WqT, g)),
                "wk": cast(_pack_w(WkT, g)),
                "wv": cast(_pack_w(WvT, g)),
                "wo": _pack_wo(WoT, g),
            }
        )

    nc = build()
    nc.finalize()
    return nc, in_maps


def kernel(**inputs):
    nc, in_maps = _prepare(inputs)
    res = run_bass_kernel_spmd(nc, in_maps, core_ids=list(range(8)))

    out = np.empty((B, S, D), np.float32)
    for b in range(B):
        t = res.results[2 * b]["yT"].astype(np.float32) + res.results[2 * b + 1][
            "yT"
        ].astype(np.float32)
        out[b] = t.transpose(1, 0, 2).reshape(1024, 2048).T
    return out
